# revision 1
# baseline (speedup 1.0000x reference)
"""Trainium2 Bass kernel for GaussianKernelLayer.

y[n] = sum_m softmax(coef)[m] * norm * exp(-0.5*|x_n - c_m|^2),
N=500000, M=256, D=4, sigma=1. Data-parallel over 8 cores (x sharded on N).

Device strategy (per core, NP=63488 padded rows, 124 chunks of 512):
  - K=16 fp16 matmul computes the FULL exp argument in PSUM:
      psum[m, n] = x.c (hi/lo split) + [ln(norm*w_m) - 0.5|c_m|^2] - 0.5|x_n|^2
    centers live on PSUM partitions (2 halves of 128), x streams as rhs.
  - -0.5|x|^2 is computed on-device (DVE square+reduce in a blocked layout),
    split hi/lo to fp16, bounced through a DRAM scratch so it can be DMA'd
    into rhs rows 14-15 in the streaming layout.
  - ACT does one big Exp per chunk: [128, 1024] PSUM -> fp16 SBUF.
  - DVE adds the two center-halves; PE reduces 128 partitions with a ones
    matmul (M=32 so a 4-chunk group fills all 128 partitions of one PSUM
    bank); DVE evacuates, DMA writes y.
"""

import math

import numpy as np

import concourse.bass as bass
import concourse.bacc as bacc_mod
import concourse.mybir as mybir
from concourse.bass_utils import run_bass_kernel_spmd
from concourse.tile import TileContext

N_CORES = 8
N_TOTAL = 500000
PER_CORE = N_TOTAL // N_CORES  # 62500
CHUNK = 512
NCHUNK = 124
NP = CHUNK * NCHUNK  # 63488 = 128 * 496
R = NP // 128  # 496
M = 256
D = 4
SIGMA = 1.0

F16 = mybir.dt.float16
F32 = mybir.dt.float32

_CACHE = {}


def _build_nc():
    nc = bacc_mod.Bacc()

    rhs_d = nc.dram_tensor("rhs", [14, NP], F16, kind="ExternalInput")
    xnat_d = nc.dram_tensor("xnat", [128, 4 * R], F32, kind="ExternalInput")
    lhsT_d = nc.dram_tensor("lhsT", [16, 256], F16, kind="ExternalInput")
    y_d = nc.dram_tensor("y", [NP], F32, kind="ExternalOutput")
    biasrow_d = nc.dram_tensor("biasrow", [2, NP], F16)  # internal scratch

    with TileContext(nc) as tc:
        with (
            tc.tile_pool(name="const", bufs=1) as constp,
            tc.tile_pool(name="pre", bufs=1) as prep,
            tc.tile_pool(name="rhsp", bufs=3) as rhsp,
            tc.tile_pool(name="expp", bufs=5) as expp,
            tc.tile_pool(name="combp", bufs=4) as combp,
            tc.tile_pool(name="ycp", bufs=3) as ycp,
            tc.tile_pool(name="psp", bufs=3, space="PSUM") as psp,
            tc.tile_pool(name="redp", bufs=2, space="PSUM") as redp,
        ):
            # --- constants ---
            lhsT_sb = constp.tile([16, 256], F16)
            nc.sync.dma_start(lhsT_sb[:], lhsT_d[:])
            ones_red = constp.tile([128, 32], F16)
            nc.vector.memset(ones_red[:], 1.0)

            # --- preamble: bias rows = -0.5*|x|^2 in fp16 hi/lo ---
            xn = prep.tile([128, 4 * R], F32)
            nc.sync.dma_start(xn[:], xnat_d[:])
            sq = prep.tile([128, 4 * R], F32)
            nc.vector.tensor_tensor(sq[:], xn[:], xn[:], mybir.AluOpType.mult)
            s = prep.tile([128, R], F32)
            nc.vector.tensor_reduce(
                s[:],
                sq[:].rearrange("p (f d) -> p f d", d=4),
                axis=mybir.AxisListType.X,
                op=mybir.AluOpType.add,
            )
            sh = prep.tile([128, R], F32)
            nc.vector.tensor_scalar_mul(sh[:], s[:], -0.5)
            bp = prep.tile([128, 2 * R], F16)
            nc.vector.tensor_copy(bp[:, 0:R], sh[:])
            # (bias_hi * -1) + sh = sh - bias_hi
            nc.vector.scalar_tensor_tensor(
                bp[:, R : 2 * R],
                bp[:, 0:R],
                -1.0,
                sh[:],
                mybir.AluOpType.mult,
                mybir.AluOpType.add,
            )
            # funnel in 4 partition-quarters so early chunks only wait on the
            # first quarter: partitions 32q..32q+32 hold n in [q*NP/4, ...)
            NQ = NP // 4
            for fq in range(4):
                nc.sync.dma_start(
                    biasrow_d[:, fq * NQ : (fq + 1) * NQ].rearrange(
                        "t (p f) -> p t f", p=32
                    ),
                    bp[32 * fq : 32 * fq + 32, :].rearrange("p (t f) -> p t f", t=2),
                )

            # --- main loop: groups of G chunks share one rhs DMA pair ---
            G = 8
            rp = None
            for g0 in range(0, NCHUNK, G):
                gsz = min(G, NCHUNK - g0)
                rhs_t = rhsp.tile([16, G * CHUNK], F16, tag="rhs")
                nc.sync.dma_start(
                    rhs_t[0:14, 0 : gsz * CHUNK],
                    rhs_d[:, g0 * CHUNK : (g0 + gsz) * CHUNK],
                )
                nc.sync.dma_start(
                    rhs_t[14:16, 0 : gsz * CHUNK],
                    biasrow_d[:, g0 * CHUNK : (g0 + gsz) * CHUNK],
                )
                for kk in range(gsz):
                    k = g0 + kk
                    rcol = kk * CHUNK
                    ps = psp.tile([128, 2 * CHUNK], F32, tag="ps")
                    nc.tensor.matmul(
                        ps[:, 0:CHUNK],
                        lhsT_sb[:, 0:128],
                        rhs_t[:, rcol : rcol + CHUNK],
                        start=True,
                        stop=True,
                    )
                    nc.tensor.matmul(
                        ps[:, CHUNK : 2 * CHUNK],
                        lhsT_sb[:, 128:256],
                        rhs_t[:, rcol : rcol + CHUNK],
                        start=True,
                        stop=True,
                    )

                    ex = expp.tile([128, 2 * CHUNK], F16, tag="ex")
                    nc.scalar.activation(
                        ex[:], ps[:], mybir.ActivationFunctionType.Exp
                    )

                    cb = combp.tile([128, CHUNK], F16, tag="cb")
                    nc.vector.tensor_tensor(
                        cb[:], ex[:, 0:CHUNK], ex[:, CHUNK : 2 * CHUNK],
                        mybir.AluOpType.add,
                    )

                    q = k % 4
                    if q == 0:
                        rp = redp.tile([128, CHUNK], F32, tag="rp")
                    nc.tensor.matmul(
                        rp[32 * q : 32 * q + 32, :],
                        ones_red[:],
                        cb[:],
                        start=True,
                        stop=True,
                        tile_position=(0, 32 * q),
                    )

                    if q == 3:
                        j = k // 4
                        yc = ycp.tile([128, CHUNK], F32, tag="yc")
                        nc.vector.tensor_copy(yc[:], rp[:])
                        nc.sync.dma_start(
                            y_d[4 * j * CHUNK : (4 * j + 4) * CHUNK].rearrange(
                                "(p f) -> p f", p=4
                            ),
                            yc[0:97:32, :],
                        )
    nc.compile()
    return nc


def _host_prep(x, centers, coefficients):
    """Small host-side prep: softmax over 256 coefficients, center hi/lo
    split, per-center bias. All O(M) except the per-core x layout work."""
    x = np.ascontiguousarray(np.asarray(x, dtype=np.float32))
    centers = np.asarray(centers, dtype=np.float32)
    coefficients = np.asarray(coefficients, dtype=np.float32)

    norm_const = np.float32(1.0 / ((2.0 * math.pi) ** (D / 2) * SIGMA**D))
    e = np.exp(coefficients - coefficients.max())
    w = (e / e.sum()).astype(np.float32)
    b = np.log(w * norm_const).astype(np.float32) - 0.5 * (centers**2).sum(axis=1)

    cT = centers.T  # [4, 256]
    c_hi = cT.astype(np.float16)
    c_lo = (cT - c_hi.astype(np.float32)).astype(np.float16)
    b_hi = b.astype(np.float16)
    b_lo = (b - b_hi.astype(np.float32)).astype(np.float16)

    lhsT = np.empty((16, 256), dtype=np.float16)
    lhsT[0:4] = c_hi
    lhsT[4:8] = c_hi
    lhsT[8:12] = c_lo
    lhsT[12] = b_hi
    lhsT[13] = b_lo
    lhsT[14] = 1.0
    lhsT[15] = 1.0

    in_maps = []
    for i in range(N_CORES):
        xs = x[i * PER_CORE : (i + 1) * PER_CORE]
        xp = np.zeros((NP, D), dtype=np.float32)
        xp[:PER_CORE] = xs
        xh = xp.astype(np.float16)
        xl = (xp - xh.astype(np.float32)).astype(np.float16)
        rhs = np.empty((14, NP), dtype=np.float16)
        rhs[0:4] = xh.T
        rhs[4:8] = xl.T
        rhs[8:12] = xh.T
        rhs[12] = 1.0
        rhs[13] = 1.0
        xnat = np.ascontiguousarray(xp.reshape(128, R * D))
        in_maps.append({"rhs": rhs, "xnat": xnat, "lhsT": lhsT.copy()})
    return in_maps


last_result = None


def kernel(x, centers, coefficients):
    global last_result
    if "nc" not in _CACHE:
        _CACHE["nc"] = _build_nc()
    nc = _CACHE["nc"]
    in_maps = _host_prep(x, centers, coefficients)
    res = run_bass_kernel_spmd(nc, in_maps, core_ids=list(range(N_CORES)))
    last_result = res
    y = np.concatenate([r["y"][:PER_CORE] for r in res.results])
    return y.astype(np.float32)



# revision 2
# speedup vs baseline: 1.1203x; 1.1203x over previous
"""Trainium2 Bass kernel for GaussianKernelLayer.

y[n] = sum_m softmax(coef)[m] * norm * exp(-0.5*|x_n - c_m|^2),
N=500000, M=256, D=4, sigma=1. Data-parallel over 8 cores (x sharded on N).

v2 design (per core, NP=63488 padded rows, 124 chunks of 512):
  - All exponent-argument assembly happens in ONE K=16 fp16 matmul per
    m-half, in log2 units:
      psum[m, n] = log2e*(x.c - 0.5|x|^2) + [log2(w_m*norm) -
                   0.5*log2e*|c_m|^2 + S]  =: t'  (so gauss = 2^t')
    Centers are the stationary lhsT; the two m-halves live in different
    PE row-groups (partitions 0-15 and 32-47) so their matmuls run
    concurrently in the 128x128 array.
  - exp is split across two engines, alternating by chunk:
      ACT: activation(Exp, scale=ln2) -> exact 2^t' in fp16
      DVE: one tensor_scalar: uint16(t'*1024 + (15*1024 - C)) whose bit
           pattern IS fp16(2^t') up to a piecewise-linear mantissa
           (Schraudolph); C=60 tuned for min L2 error (~3e-3).
  - Reduction over m: ones-matmul, K=128, accumulating both halves into
    one PSUM quadrant (col-group q=k%4), so no separate half-combine.
  - y evacuation PSUM->SBUF alternates ACT/DVE copies; DMA to DRAM.
  - Host does the O(M) prep + fp16 hi/lo splits and the final 2^-S scale.
"""

import math

import numpy as np

import concourse.bass as bass
import concourse.bacc as bacc_mod
import concourse.mybir as mybir
from concourse.bass_utils import run_bass_kernel_spmd
from concourse.tile import TileContext

N_CORES = 8
N_TOTAL = 500000
PER_CORE = N_TOTAL // N_CORES  # 62500
CHUNK = 512
NCHUNK = 124
NP = CHUNK * NCHUNK  # 63488
M = 256
D = 4
SIGMA = 1.0

F16 = mybir.dt.float16
F32 = mybir.dt.float32
U16 = mybir.dt.uint16

LOG2E = 1.0 / math.log(2.0)
LN2 = math.log(2.0)
SCH_C = 60.0  # Schraudolph shift, tuned on host sim
SCH_BIAS = float(15 * 1024 - SCH_C)

# chunk -> engine for the exp pass. ACT is a bit faster per element than
# the DVE Schraudolph (1.2 vs 0.96 GHz), so give ACT 5 of every 9.
ACT_PATTERN = (1, 0, 1, 0, 1, 0, 1, 1, 0)

G = 8  # chunks per rhs DMA group

_CACHE = {}


def _build_nc():
    nc = bacc_mod.Bacc()

    rhs_d = nc.dram_tensor("rhs", [16, NP], F16, kind="ExternalInput")
    lhsT_d = nc.dram_tensor("lhsT", [2, 16, 128], F16, kind="ExternalInput")
    y_d = nc.dram_tensor("y", [NP], F32, kind="ExternalOutput")

    with TileContext(nc) as tc:
        with (
            tc.tile_pool(name="const", bufs=1) as constp,
            tc.tile_pool(name="rhsp", bufs=3) as rhsp,
            tc.tile_pool(name="cbp", bufs=4) as cbp,
            tc.tile_pool(name="ycp", bufs=3) as ycp,
            tc.tile_pool(name="psp", bufs=3, space="PSUM") as psp,
            tc.tile_pool(name="yp", bufs=2, space="PSUM") as yp,
        ):
            # --- constants ---
            lhsT_sb = constp.tile([48, 128], F16)
            nc.sync.dma_start(lhsT_sb[0:16, :], lhsT_d[0])
            nc.sync.dma_start(lhsT_sb[32:48, :], lhsT_d[1])
            ones_red = constp.tile([128, 32], F16)
            nc.vector.memset(ones_red[:], 1.0)

            yps = None
            for g0 in range(0, NCHUNK, G):
                gsz = min(G, NCHUNK - g0)
                rhs_t = rhsp.tile([48, G * CHUNK], F16, tag="rhs")
                nc.sync.dma_start(
                    rhs_t[0:16, 0 : gsz * CHUNK],
                    rhs_d[:, g0 * CHUNK : (g0 + gsz) * CHUNK],
                )
                nc.sync.dma_start(
                    rhs_t[32:48, 0 : gsz * CHUNK],
                    rhs_d[:, g0 * CHUNK : (g0 + gsz) * CHUNK],
                )
                for kk in range(gsz):
                    k = g0 + kk
                    col = kk * CHUNK
                    ps = psp.tile([128, 2 * CHUNK], F32, tag="ps")
                    nc.tensor.matmul(
                        ps[:, 0:CHUNK],
                        lhsT_sb[0:16, :],
                        rhs_t[0:16, col : col + CHUNK],
                        start=True,
                        stop=True,
                    )
                    nc.tensor.matmul(
                        ps[:, CHUNK : 2 * CHUNK],
                        lhsT_sb[32:48, :],
                        rhs_t[32:48, col : col + CHUNK],
                        start=True,
                        stop=True,
                    )

                    if ACT_PATTERN[k % len(ACT_PATTERN)]:
                        cb = cbp.tile([128, 2 * CHUNK], F16, tag="cb")
                        nc.scalar.activation(
                            cb[:],
                            ps[:],
                            mybir.ActivationFunctionType.Exp,
                            scale=LN2,
                        )
                        cbf = cb[:]
                    else:
                        cb = cbp.tile([128, 2 * CHUNK], U16, tag="cb")
                        nc.vector.tensor_scalar(
                            cb[:],
                            ps[:],
                            1024.0,
                            SCH_BIAS,
                            mybir.AluOpType.mult,
                            mybir.AluOpType.add,
                        )
                        cbf = cb[:].bitcast(F16)

                    q = k % 4
                    if q == 0:
                        yps = yp.tile([128, CHUNK], F32, tag="yps")
                    nc.tensor.matmul(
                        yps[32 * q : 32 * q + 32, :],
                        ones_red[:],
                        cbf[:, 0:CHUNK],
                        start=True,
                        stop=False,
                        tile_position=(0, 32 * q),
                    )
                    nc.tensor.matmul(
                        yps[32 * q : 32 * q + 32, :],
                        ones_red[:],
                        cbf[:, CHUNK : 2 * CHUNK],
                        start=False,
                        stop=True,
                        tile_position=(0, 32 * q),
                    )

                    if q == 3:
                        j = k // 4
                        yc = ycp.tile([128, CHUNK], F32, tag="yc")
                        if j % 2 == 0:
                            nc.scalar.copy(yc[:], yps[:])
                        else:
                            nc.vector.tensor_copy(yc[:], yps[:])
                        nc.sync.dma_start(
                            y_d[4 * j * CHUNK : (4 * j + 4) * CHUNK].rearrange(
                                "(p f) -> p f", p=4
                            ),
                            yc[0:97:32, :],
                        )
    nc.compile()
    return nc


def _host_prep(x, centers, coefficients):
    """O(M) center prep + per-core x layout, all in log2 units."""
    x = np.ascontiguousarray(np.asarray(x, dtype=np.float32))
    centers = np.asarray(centers, dtype=np.float32)
    coefficients = np.asarray(coefficients, dtype=np.float32)

    norm_const = np.float32(1.0 / ((2.0 * math.pi) ** (D / 2) * SIGMA**D))
    e = np.exp(coefficients - coefficients.max())
    w = (e / e.sum()).astype(np.float32)

    s = np.float32(math.sqrt(LOG2E))
    b = centers.T * s  # [4, 256]
    b_hi = b.astype(np.float16)
    b_lo = (b - b_hi.astype(np.float32)).astype(np.float16)

    g_raw = (
        np.log2(w * norm_const) - 0.5 * LOG2E * (centers**2).sum(axis=1)
    ).astype(np.float32)
    S = np.float32(math.floor(12.0 - np.log2(w * norm_const).max()))
    g = g_raw + S
    g_hi = g.astype(np.float16)
    g_lo = (g - g_hi.astype(np.float32)).astype(np.float16)

    lhsT = np.empty((2, 16, 128), dtype=np.float16)
    for h in range(2):
        sl = slice(128 * h, 128 * (h + 1))
        lhsT[h, 0:4] = b_hi[:, sl]
        lhsT[h, 4:8] = b_hi[:, sl]
        lhsT[h, 8:12] = b_lo[:, sl]
        lhsT[h, 12] = 1.0
        lhsT[h, 13] = 1.0
        lhsT[h, 14] = g_hi[sl]
        lhsT[h, 15] = g_lo[sl]

    in_maps = []
    for i in range(N_CORES):
        xs = x[i * PER_CORE : (i + 1) * PER_CORE]
        xp = np.zeros((NP, D), dtype=np.float32)
        xp[:PER_CORE] = xs
        a = xp * s
        a_hi = a.astype(np.float16)
        a_lo = (a - a_hi.astype(np.float32)).astype(np.float16)
        hbias = (-0.5 * LOG2E * (xp**2).sum(axis=1)).astype(np.float32)
        h_hi = hbias.astype(np.float16)
        h_lo = (hbias - h_hi.astype(np.float32)).astype(np.float16)
        rhs = np.empty((16, NP), dtype=np.float16)
        rhs[0:4] = a_hi.T
        rhs[4:8] = a_lo.T
        rhs[8:12] = a_hi.T
        rhs[12] = h_hi
        rhs[13] = h_lo
        rhs[14] = 1.0
        rhs[15] = 1.0
        in_maps.append({"rhs": rhs, "lhsT": lhsT.copy()})
    return in_maps, float(S)


last_result = None


def kernel(x, centers, coefficients):
    global last_result
    if "nc" not in _CACHE:
        _CACHE["nc"] = _build_nc()
    nc = _CACHE["nc"]
    in_maps, S = _host_prep(x, centers, coefficients)
    res = run_bass_kernel_spmd(nc, in_maps, core_ids=list(range(N_CORES)))
    last_result = res
    y = np.concatenate([r["y"][:PER_CORE] for r in res.results])
    return (y * np.float32(2.0 ** (-S))).astype(np.float32)


# revision 3
# speedup vs baseline: 1.1862x; 1.0588x over previous
"""Trainium2 Bass kernel for GaussianKernelLayer.

y[n] = sum_m softmax(coef)[m] * norm * exp(-0.5*|x_n - c_m|^2),
N=500000, M=256, D=4, sigma=1. Data-parallel over 8 cores (x sharded on N).

v3 design (per core, NP=63488 padded rows, 124 chunks of 512):
  - One K=16 fp16 matmul per m-half assembles the full exp argument in
    log2 units directly in PSUM:
      psum[m, n] = log2e*(x.c - 0.5|x|^2) + [log2(w_m*norm)
                   - 0.5*log2e*|c_m|^2 + S]  =: t'   (gauss = 2^t')
    The two m-halves sit in different PE row-groups (partitions 0-15 /
    32-47) so their matmuls execute concurrently in the 128x128 array.
  - exp split across two engines, alternating by chunk:
      ACT: activation(Exp, scale=ln2) -> exact 2^t' (fp16)
      DVE: Schraudolph in ONE tensor_scalar: uint16(t'*1024 + 15*1024-C)
           whose bit pattern IS fp16(2^t') (piecewise-linear mantissa).
  - Reduce over m via ones-matmul (K=128) into PSUM col-group q=k%4;
    for a fraction of chunks DVE pre-adds the halves so only one
    reduce matmul is needed (PE/DVE load balance knob).
  - The reduce matmuls are software-pipelined D chunks behind the main
    matmuls so they never head-of-line block the in-order PE queue ->
    the PE stays continuously busy and the HAM clock-gate reaches 2.4
    GHz. ~10 warm-up matmuls on memset data run during the first DMA.
  - Host does O(M) prep, fp16 hi/lo splits, and the final 2^-S scale.
"""

import math

import numpy as np

import concourse.bass as bass
import concourse.bacc as bacc_mod
import concourse.mybir as mybir
from concourse.bass_utils import run_bass_kernel_spmd
from concourse.tile import TileContext

N_CORES = 8
N_TOTAL = 500000
PER_CORE = N_TOTAL // N_CORES  # 62500
CHUNK = 512
NCHUNK = 124
NP = CHUNK * NCHUNK  # 63488
M = 256
D = 4
SIGMA = 1.0

F16 = mybir.dt.float16
F32 = mybir.dt.float32
U16 = mybir.dt.uint16

LOG2E = 1.0 / math.log(2.0)
LN2 = math.log(2.0)
SCH_C = 60.0  # Schraudolph shift, tuned on host sim
SCH_BIAS = float(15 * 1024 - SCH_C)

# chunk -> exp engine (1 = ACT exact exp, 0 = DVE Schraudolph)
ACT_PATTERN = (1, 0, 1, 0, 1, 0, 1, 1, 0)
# chunk -> pre-combine halves on DVE (1) vs two reduce matmuls (0)
COMB_PATTERN = (1, 0)

PIPE_D = 4  # reduce stage lags the matmul stage by this many chunks
WARMUP_MM = 10
G = 8  # chunks per rhs DMA group

_CACHE = {}


def _build_nc():
    nc = bacc_mod.Bacc()

    rhs_d = nc.dram_tensor("rhs", [16, NP], F16, kind="ExternalInput")
    lhsT_d = nc.dram_tensor("lhsT", [2, 16, 128], F16, kind="ExternalInput")
    y_d = nc.dram_tensor("y", [NP], F32, kind="ExternalOutput")

    with TileContext(nc) as tc:
        with (
            tc.tile_pool(name="const", bufs=1) as constp,
            tc.tile_pool(name="rhsp", bufs=3) as rhsp,
            tc.tile_pool(name="cbp", bufs=PIPE_D + 3) as cbp,
            tc.tile_pool(name="cb2p", bufs=PIPE_D + 3) as cb2p,
            tc.tile_pool(name="ycp", bufs=3) as ycp,
            tc.tile_pool(name="psp", bufs=3, space="PSUM") as psp,
            tc.tile_pool(name="yp", bufs=2, space="PSUM") as yp,
        ):
            # --- constants ---
            lhsT_sb = constp.tile([48, 128], F16)
            nc.sync.dma_start(lhsT_sb[0:16, :], lhsT_d[0])
            nc.sync.dma_start(lhsT_sb[32:48, :], lhsT_d[1])
            ones_red = constp.tile([128, 32], F16)
            nc.vector.memset(ones_red[:], 1.0)
            scratch = constp.tile([16, CHUNK], F16)
            nc.vector.memset(scratch[:], 0.0)

            # --- HAM warm-up: dummy matmuls on memset data (no DMA dep) ---
            ps_w = psp.tile([128, 2 * CHUNK], F32, tag="ps")
            for _ in range(WARMUP_MM):
                nc.tensor.matmul(
                    ps_w[:, 0:CHUNK],
                    scratch[:, 0:128],
                    scratch[:, 0:CHUNK],
                    start=True,
                    stop=True,
                )

            cbs = {}  # k -> (cb_f16_ap, combined)
            yps = None

            def reduce_stage(k):
                cbf, combined = cbs.pop(k)
                q = k % 4
                nonlocal yps
                if q == 0:
                    yps = yp.tile([128, CHUNK], F32, tag="yps")
                if combined:
                    nc.tensor.matmul(
                        yps[32 * q : 32 * q + 32, :],
                        ones_red[:],
                        cbf[:, 0:CHUNK],
                        start=True,
                        stop=True,
                        tile_position=(0, 32 * q),
                    )
                else:
                    nc.tensor.matmul(
                        yps[32 * q : 32 * q + 32, :],
                        ones_red[:],
                        cbf[:, 0:CHUNK],
                        start=True,
                        stop=False,
                        tile_position=(0, 32 * q),
                    )
                    nc.tensor.matmul(
                        yps[32 * q : 32 * q + 32, :],
                        ones_red[:],
                        cbf[:, CHUNK : 2 * CHUNK],
                        start=False,
                        stop=True,
                        tile_position=(0, 32 * q),
                    )
                if q == 3:
                    j = k // 4
                    yc = ycp.tile([128, CHUNK], F32, tag="yc")
                    if j % 2 == 0:
                        nc.scalar.copy(yc[:], yps[:])
                    else:
                        nc.vector.tensor_copy(yc[:], yps[:])
                    nc.sync.dma_start(
                        y_d[4 * j * CHUNK : (4 * j + 4) * CHUNK].rearrange(
                            "(p f) -> p f", p=4
                        ),
                        yc[0:97:32, :],
                    )

            for g0 in range(0, NCHUNK, G):
                gsz = min(G, NCHUNK - g0)
                rhs_t = rhsp.tile([48, G * CHUNK], F16, tag="rhs")
                nc.sync.dma_start(
                    rhs_t[0:16, 0 : gsz * CHUNK],
                    rhs_d[:, g0 * CHUNK : (g0 + gsz) * CHUNK],
                )
                nc.sync.dma_start(
                    rhs_t[32:48, 0 : gsz * CHUNK],
                    rhs_d[:, g0 * CHUNK : (g0 + gsz) * CHUNK],
                )
                for kk in range(gsz):
                    k = g0 + kk
                    col = kk * CHUNK
                    ps = psp.tile([128, 2 * CHUNK], F32, tag="ps")
                    nc.tensor.matmul(
                        ps[:, 0:CHUNK],
                        lhsT_sb[0:16, :],
                        rhs_t[0:16, col : col + CHUNK],
                        start=True,
                        stop=True,
                    )
                    nc.tensor.matmul(
                        ps[:, CHUNK : 2 * CHUNK],
                        lhsT_sb[32:48, :],
                        rhs_t[32:48, col : col + CHUNK],
                        start=True,
                        stop=True,
                    )

                    if ACT_PATTERN[k % len(ACT_PATTERN)]:
                        cb = cbp.tile([128, 2 * CHUNK], F16, tag="cb")
                        nc.scalar.activation(
                            cb[:],
                            ps[:],
                            mybir.ActivationFunctionType.Exp,
                            scale=LN2,
                        )
                        cbf = cb[:]
                    else:
                        cb = cbp.tile([128, 2 * CHUNK], U16, tag="cb")
                        nc.vector.tensor_scalar(
                            cb[:],
                            ps[:],
                            1024.0,
                            SCH_BIAS,
                            mybir.AluOpType.mult,
                            mybir.AluOpType.add,
                        )
                        cbf = cb[:].bitcast(F16)

                    if COMB_PATTERN[k % len(COMB_PATTERN)]:
                        cb2 = cb2p.tile([128, CHUNK], F16, tag="cb2")
                        nc.vector.tensor_tensor(
                            cb2[:],
                            cbf[:, 0:CHUNK],
                            cbf[:, CHUNK : 2 * CHUNK],
                            mybir.AluOpType.add,
                        )
                        cbs[k] = (cb2[:], True)
                    else:
                        cbs[k] = (cbf, False)

                    if k >= PIPE_D:
                        reduce_stage(k - PIPE_D)

            for k in range(NCHUNK - PIPE_D, NCHUNK):
                reduce_stage(k)
    nc.compile()
    return nc


def _host_prep(x, centers, coefficients):
    """O(M) center prep + per-core x layout, all in log2 units."""
    x = np.ascontiguousarray(np.asarray(x, dtype=np.float32))
    centers = np.asarray(centers, dtype=np.float32)
    coefficients = np.asarray(coefficients, dtype=np.float32)

    norm_const = np.float32(1.0 / ((2.0 * math.pi) ** (D / 2) * SIGMA**D))
    e = np.exp(coefficients - coefficients.max())
    w = (e / e.sum()).astype(np.float32)

    s = np.float32(math.sqrt(LOG2E))
    b = centers.T * s  # [4, 256]
    b_hi = b.astype(np.float16)
    b_lo = (b - b_hi.astype(np.float32)).astype(np.float16)

    g_raw = (
        np.log2(w * norm_const) - 0.5 * LOG2E * (centers**2).sum(axis=1)
    ).astype(np.float32)
    S = np.float32(math.floor(12.0 - np.log2(w * norm_const).max()))
    g = g_raw + S
    g_hi = g.astype(np.float16)
    g_lo = (g - g_hi.astype(np.float32)).astype(np.float16)

    lhsT = np.empty((2, 16, 128), dtype=np.float16)
    for h in range(2):
        sl = slice(128 * h, 128 * (h + 1))
        lhsT[h, 0:4] = b_hi[:, sl]
        lhsT[h, 4:8] = b_hi[:, sl]
        lhsT[h, 8:12] = b_lo[:, sl]
        lhsT[h, 12] = 1.0
        lhsT[h, 13] = 1.0
        lhsT[h, 14] = g_hi[sl]
        lhsT[h, 15] = g_lo[sl]

    in_maps = []
    for i in range(N_CORES):
        xs = x[i * PER_CORE : (i + 1) * PER_CORE]
        xp = np.zeros((NP, D), dtype=np.float32)
        xp[:PER_CORE] = xs
        a = xp * s
        a_hi = a.astype(np.float16)
        a_lo = (a - a_hi.astype(np.float32)).astype(np.float16)
        hbias = (-0.5 * LOG2E * (xp**2).sum(axis=1)).astype(np.float32)
        h_hi = hbias.astype(np.float16)
        h_lo = (hbias - h_hi.astype(np.float32)).astype(np.float16)
        rhs = np.empty((16, NP), dtype=np.float16)
        rhs[0:4] = a_hi.T
        rhs[4:8] = a_lo.T
        rhs[8:12] = a_hi.T
        rhs[12] = h_hi
        rhs[13] = h_lo
        rhs[14] = 1.0
        rhs[15] = 1.0
        in_maps.append({"rhs": rhs, "lhsT": lhsT.copy()})
    return in_maps, float(S)


last_result = None


def kernel(x, centers, coefficients):
    global last_result
    if "nc" not in _CACHE:
        _CACHE["nc"] = _build_nc()
    nc = _CACHE["nc"]
    in_maps, S = _host_prep(x, centers, coefficients)
    res = run_bass_kernel_spmd(nc, in_maps, core_ids=list(range(N_CORES)))
    last_result = res
    y = np.concatenate([r["y"][:PER_CORE] for r in res.results])
    return (y * np.float32(2.0 ** (-S))).astype(np.float32)


# revision 10
# speedup vs baseline: 1.4282x; 1.2040x over previous
"""Trainium2 Bass kernel for GaussianKernelLayer.

y[n] = sum_m softmax(coef)[m] * norm * exp(-0.5*|x_n - c_m|^2),
N=500000, M=256, D=4, sigma=1. Data-parallel over 8 cores (x sharded on N).

v3 design (per core, NP=63488 padded rows, 124 chunks of 512):
  - One K=16 fp16 matmul per m-half assembles the full exp argument in
    log2 units directly in PSUM:
      psum[m, n] = log2e*(x.c - 0.5|x|^2) + [log2(w_m*norm)
                   - 0.5*log2e*|c_m|^2 + S]  =: t'   (gauss = 2^t')
    The two m-halves sit in different PE row-groups (partitions 0-15 /
    32-47) so their matmuls execute concurrently in the 128x128 array.
  - exp split across two engines, alternating by chunk:
      ACT: activation(Exp, scale=ln2) -> exact 2^t' (fp16)
      DVE: Schraudolph in ONE tensor_scalar: uint16(t'*1024 + 15*1024-C)
           whose bit pattern IS fp16(2^t') (piecewise-linear mantissa).
  - Reduce over m via ones-matmul (K=128) into PSUM col-group q=k%4;
    for a fraction of chunks DVE pre-adds the halves so only one
    reduce matmul is needed (PE/DVE load balance knob).
  - The reduce matmuls are software-pipelined D chunks behind the main
    matmuls so they never head-of-line block the in-order PE queue ->
    the PE stays continuously busy and the HAM clock-gate reaches 2.4
    GHz. ~10 warm-up matmuls on memset data run during the first DMA.
  - Host does O(M) prep, fp16 hi/lo splits, and the final 2^-S scale.
"""

import math

import numpy as np

import concourse.bass as bass
import concourse.bacc as bacc_mod
import concourse.mybir as mybir
from concourse.bass_utils import run_bass_kernel_spmd
from concourse.tile import TileContext

N_CORES = 8
N_TOTAL = 500000
PER_CORE = N_TOTAL // N_CORES  # 62500
CHUNK = 512
NCHUNK = 124
NP = CHUNK * NCHUNK  # 63488
M = 256
D = 4
SIGMA = 1.0

F16 = mybir.dt.float16
F32 = mybir.dt.float32
U16 = mybir.dt.uint16

LOG2E = 1.0 / math.log(2.0)
LN2 = math.log(2.0)
SCH_C = 60.0  # Schraudolph shift, tuned on host sim
SCH_BIAS = float(15 * 1024 - SCH_C)

# chunk -> exp engine (1 = ACT exact exp, 0 = DVE Schraudolph)
ACT_PATTERN = (1, 0, 1, 0, 1, 0, 1, 1, 0)
# chunk -> pre-combine halves on DVE (1) vs two reduce matmuls (0)
COMB_PATTERN = (1, 0)

PIPE_D = 4  # reduce stage lags the matmul stage by this many chunks
WARMUP_MM = 14
GP = 4  # chunk-PAIRS per rhs DMA group (8 chunks)
NPAIR = NCHUNK // 2  # 62
NGROUP = (NPAIR + GP - 1) // GP  # 16

_CACHE = {}


def _build_nc():
    nc = bacc_mod.Bacc()

    # rhs packed per group of GP chunk-pairs: 64 partition-lines = {A-chunk
    # rows, A rows again, B rows, B rows again} x (pair cols side by side),
    # so ONE 3-dim DMA fills the 4 PE row-group bands for 2*GP chunks.
    rhs_d = nc.dram_tensor("rhs", [NGROUP, 64, GP * CHUNK], F16, kind="ExternalInput")
    lhsT_d = nc.dram_tensor("lhsT", [2, 16, 128], F16, kind="ExternalInput")
    y_d = nc.dram_tensor("y", [NP], F32, kind="ExternalOutput")

    with TileContext(nc) as tc:
        with (
            tc.tile_pool(name="const", bufs=1) as constp,
            tc.tile_pool(name="rhsp", bufs=4) as rhsp,
            tc.tile_pool(name="cbp", bufs=PIPE_D + 3) as cbp,
            tc.tile_pool(name="cb2p", bufs=PIPE_D + 3) as cb2p,
            tc.tile_pool(name="ycp", bufs=3) as ycp,
            tc.tile_pool(name="psp", bufs=3, space="PSUM") as psp,
            tc.tile_pool(name="yp", bufs=2, space="PSUM") as yp,
        ):
            # --- constants: half0/half1 weights at all four row-groups ---
            lhsT_sb = constp.tile([112, 128], F16)
            nc.sync.dma_start(lhsT_sb[0:16, :], lhsT_d[0])
            nc.sync.dma_start(lhsT_sb[32:48, :], lhsT_d[1])
            nc.sync.dma_start(lhsT_sb[64:80, :], lhsT_d[0])
            nc.sync.dma_start(lhsT_sb[96:112, :], lhsT_d[1])
            ones_red = constp.tile([128, 32], F16)
            nc.vector.memset(ones_red[:], 1.0)
            scratch = constp.tile([16, CHUNK], F16)
            nc.vector.memset(scratch[:], 0.0)

            # --- HAM warm-up: dummy matmuls on memset data (no DMA dep) ---
            ps_w = psp.tile([128, 2 * CHUNK], F32, tag="ps")
            for _ in range(WARMUP_MM):
                nc.tensor.matmul(
                    ps_w[:, 0:CHUNK],
                    scratch[:, 0:128],
                    scratch[:, 0:CHUNK],
                    start=True,
                    stop=True,
                )

            cbs = {}  # k -> (cb_f16_ap, combined)
            yps = None

            def reduce_stage(k):
                cbf, combined = cbs.pop(k)
                q = k % 4
                nonlocal yps
                if q == 0:
                    yps = yp.tile([128, CHUNK], F32, tag="yps")
                if combined:
                    nc.tensor.matmul(
                        yps[32 * q : 32 * q + 32, :],
                        ones_red[:],
                        cbf[:, 0:CHUNK],
                        start=True,
                        stop=True,
                        tile_position=(0, 32 * q),
                    )
                else:
                    nc.tensor.matmul(
                        yps[32 * q : 32 * q + 32, :],
                        ones_red[:],
                        cbf[:, 0:CHUNK],
                        start=True,
                        stop=False,
                        tile_position=(0, 32 * q),
                    )
                    nc.tensor.matmul(
                        yps[32 * q : 32 * q + 32, :],
                        ones_red[:],
                        cbf[:, CHUNK : 2 * CHUNK],
                        start=False,
                        stop=True,
                        tile_position=(0, 32 * q),
                    )
                if q == 3:
                    j = k // 4
                    yc = ycp.tile([128, CHUNK], F32, tag="yc")
                    if j % 2 == 0:
                        nc.scalar.copy(yc[:], yps[:])
                    else:
                        nc.vector.tensor_copy(yc[:], yps[:])
                    nc.gpsimd.dma_start(
                        y_d[4 * j * CHUNK : (4 * j + 4) * CHUNK].rearrange(
                            "(p f) -> p f", p=4
                        ),
                        yc[0:97:32, :],
                    )

            for p0 in range(0, NPAIR, GP):
                gi = p0 // GP
                gsz = min(GP, NPAIR - p0)
                rhs_t = rhsp.tile([128, GP * CHUNK], F16, tag="rhs")
                # four plain DMAs fill bands {0-15,32-47,64-79,96-111},
                # split across the sync and gpsimd queues
                for b, eng in ((0, nc.sync), (1, nc.gpsimd), (2, nc.sync), (3, nc.gpsimd)):
                    eng.dma_start(
                        rhs_t[32 * b : 32 * b + 16, 0 : gsz * CHUNK],
                        rhs_d[gi, 16 * b : 16 * b + 16, 0 : gsz * CHUNK],
                    )
                for kk in range(2 * gsz):
                    k = 2 * p0 + kk
                    jj = kk // 2  # pair index within group
                    half_band = 64 * (kk % 2)  # chunk A -> bands 0/32, B -> 64/96
                    col = jj * CHUNK
                    ps = psp.tile([128, 2 * CHUNK], F32, tag="ps")
                    nc.tensor.matmul(
                        ps[:, 0:CHUNK],
                        lhsT_sb[half_band : half_band + 16, :],
                        rhs_t[half_band : half_band + 16, col : col + CHUNK],
                        start=True,
                        stop=True,
                        tile_position=(half_band, 0),
                    )
                    nc.tensor.matmul(
                        ps[:, CHUNK : 2 * CHUNK],
                        lhsT_sb[half_band + 32 : half_band + 48, :],
                        rhs_t[half_band + 32 : half_band + 48, col : col + CHUNK],
                        start=True,
                        stop=True,
                        tile_position=(half_band + 32, 0),
                    )

                    if ACT_PATTERN[k % len(ACT_PATTERN)]:
                        cb = cbp.tile([128, 2 * CHUNK], F16, tag="cb")
                        nc.scalar.activation(
                            cb[:],
                            ps[:],
                            mybir.ActivationFunctionType.Exp,
                            scale=LN2,
                        )
                        cbf = cb[:]
                    else:
                        cb = cbp.tile([128, 2 * CHUNK], U16, tag="cb")
                        nc.vector.tensor_scalar(
                            cb[:],
                            ps[:],
                            1024.0,
                            SCH_BIAS,
                            mybir.AluOpType.mult,
                            mybir.AluOpType.add,
                        )
                        cbf = cb[:].bitcast(F16)

                    if COMB_PATTERN[k % len(COMB_PATTERN)]:
                        cb2 = cb2p.tile([128, CHUNK], F16, tag="cb2")
                        nc.vector.tensor_tensor(
                            cb2[:],
                            cbf[:, 0:CHUNK],
                            cbf[:, CHUNK : 2 * CHUNK],
                            mybir.AluOpType.add,
                        )
                        cbs[k] = (cb2[:], True)
                    else:
                        cbs[k] = (cbf, False)

                    if k >= PIPE_D:
                        reduce_stage(k - PIPE_D)

            for k in range(NCHUNK - PIPE_D, NCHUNK):
                reduce_stage(k)
    nc.compile()
    return nc


def _host_prep(x, centers, coefficients):
    """O(M) center prep + per-core x layout, all in log2 units."""
    x = np.ascontiguousarray(np.asarray(x, dtype=np.float32))
    centers = np.asarray(centers, dtype=np.float32)
    coefficients = np.asarray(coefficients, dtype=np.float32)

    norm_const = np.float32(1.0 / ((2.0 * math.pi) ** (D / 2) * SIGMA**D))
    e = np.exp(coefficients - coefficients.max())
    w = (e / e.sum()).astype(np.float32)

    s = np.float32(math.sqrt(LOG2E))
    b = centers.T * s  # [4, 256]
    b_hi = b.astype(np.float16)
    b_lo = (b - b_hi.astype(np.float32)).astype(np.float16)

    g_raw = (
        np.log2(w * norm_const) - 0.5 * LOG2E * (centers**2).sum(axis=1)
    ).astype(np.float32)
    S = np.float32(math.floor(12.0 - np.log2(w * norm_const).max()))
    g = g_raw + S
    g_hi = g.astype(np.float16)
    g_lo = (g - g_hi.astype(np.float32)).astype(np.float16)

    lhsT = np.empty((2, 16, 128), dtype=np.float16)
    for h in range(2):
        sl = slice(128 * h, 128 * (h + 1))
        lhsT[h, 0:4] = b_hi[:, sl]
        lhsT[h, 4:8] = b_hi[:, sl]
        lhsT[h, 8:12] = b_lo[:, sl]
        lhsT[h, 12] = 1.0
        lhsT[h, 13] = 1.0
        lhsT[h, 14] = g_hi[sl]
        lhsT[h, 15] = g_lo[sl]

    in_maps = []
    for i in range(N_CORES):
        xs = x[i * PER_CORE : (i + 1) * PER_CORE]
        xp = np.zeros((NP, D), dtype=np.float32)
        xp[:PER_CORE] = xs
        a = xp * s
        a_hi = a.astype(np.float16)
        a_lo = (a - a_hi.astype(np.float32)).astype(np.float16)
        hbias = (-0.5 * LOG2E * (xp**2).sum(axis=1)).astype(np.float32)
        h_hi = hbias.astype(np.float16)
        h_lo = (hbias - h_hi.astype(np.float32)).astype(np.float16)
        rows = np.empty((16, NP), dtype=np.float16)
        rows[0:4] = a_hi.T
        rows[4:8] = a_lo.T
        rows[8:12] = a_hi.T
        rows[12] = h_hi
        rows[13] = h_lo
        rows[14] = 1.0
        rows[15] = 1.0
        # pack per group of GP pairs: 64 lines = {A rows, A rows, B rows,
        # B rows}, pair columns side by side within a line
        rc = rows.reshape(16, NCHUNK, CHUNK).transpose(1, 0, 2)  # [124,16,512]
        rp = rc.reshape(NPAIR, 2, 16, CHUNK)
        rhs = np.zeros((NGROUP, 64, GP * CHUNK), dtype=np.float16)
        for gi in range(NGROUP):
            p0 = gi * GP
            gsz = min(GP, NPAIR - p0)
            for j in range(gsz):
                cs = slice(j * CHUNK, (j + 1) * CHUNK)
                rhs[gi, 0:16, cs] = rp[p0 + j, 0]
                rhs[gi, 16:32, cs] = rp[p0 + j, 0]
                rhs[gi, 32:48, cs] = rp[p0 + j, 1]
                rhs[gi, 48:64, cs] = rp[p0 + j, 1]
        in_maps.append({"rhs": rhs, "lhsT": lhsT.copy()})
    return in_maps, float(S)


last_result = None


def kernel(x, centers, coefficients):
    global last_result
    if "nc" not in _CACHE:
        _CACHE["nc"] = _build_nc()
    nc = _CACHE["nc"]
    in_maps, S = _host_prep(x, centers, coefficients)
    res = run_bass_kernel_spmd(nc, in_maps, core_ids=list(range(N_CORES)))
    last_result = res
    y = np.concatenate([r["y"][:PER_CORE] for r in res.results])
    return (y * np.float32(2.0 ** (-S))).astype(np.float32)


# revision 13
# speedup vs baseline: 1.6543x; 1.1583x over previous
"""Trainium2 Bass kernel for GaussianKernelLayer.

y[n] = sum_m softmax(coef)[m] * norm * exp(-0.5*|x_n - c_m|^2),
N=500000, M=256, D=4, sigma=1. Data-parallel over 8 cores (x sharded on N).

v5 design (per core, NP=63488 padded rows, 124 chunks of 512):
  - One K=32 fp16 matmul per (chunk, m-half) assembles the full exp
    argument in log2 units directly in PSUM:
      psum[m, n] = log2e*(x.c - 0.5|x|^2) + [log2(w_m*norm)
                   - 0.5*log2e*|c_m|^2 + S]  =: t'   (gauss = 2^t')
    K is padded 16->32 with zero weight rows: the TRN2 HAM clock-gate
    only un-throttles the PE to 2.4 GHz when the 128x128 array is
    near-fully active, and 4 concurrent K=32 row-group tiles qualify
    (4 dense K=16 streams never warm up - measured).
  - Matmuls are issued in QUADS of adjacent instructions that target
    the 4 row-groups (mains: 2 chunks x 2 m-halves) or the 4
    col-groups (ones-reduce over m, K=128) -> each quad executes
    concurrently in ~215 ns warm (measured 209-211 ns/quad).
  - exp split across two engines, alternating by chunk:
      ACT: activation(Exp, scale=ln2) -> exact 2^t' (fp16)
      DVE: Schraudolph in ONE tensor_scalar: uint16(t'*1024+15*1024-C),
           the bit pattern IS fp16(2^t') (piecewise-linear mantissa).
  - Reduce stage is software-pipelined D chunks behind the mains so the
    in-order PE queue never stalls; rhs DMAs land in 4 row-group bands
    of static SBUF buffers (64 partition-lines per group DMA).
  - Host does O(M) prep, fp16 hi/lo splits, and the final 2^-S scale.
"""

import math

import numpy as np

import concourse.bass as bass
import concourse.bacc as bacc_mod
import concourse.mybir as mybir
from concourse.bass_utils import run_bass_kernel_spmd
from concourse.tile import TileContext

N_CORES = 8
N_TOTAL = 500000
PER_CORE = N_TOTAL // N_CORES  # 62500
CHUNK = 512
NCHUNK = 124
NP = CHUNK * NCHUNK  # 63488
M = 256
D = 4
SIGMA = 1.0

F16 = mybir.dt.float16
F32 = mybir.dt.float32
U16 = mybir.dt.uint16

LOG2E = 1.0 / math.log(2.0)
LN2 = math.log(2.0)
SCH_C = 60.0  # Schraudolph shift, tuned on host sim
SCH_BIAS = float(15 * 1024 - SCH_C)

# chunk -> exp engine (1 = ACT exact exp, 0 = DVE Schraudolph)
ACT_PATTERN = (1, 0, 1, 0, 1, 0, 1, 1, 0)

PIPE_D = 6  # reduce stage lags the matmul stage by this many chunks
WARMUP_QUADS = 0
GP = 4  # chunk-PAIRS per rhs DMA group (8 chunks)
NPAIR = NCHUNK // 2  # 62
NGROUP = (NPAIR + GP - 1) // GP  # 16
NRHSBUF = 4

_CACHE = {}


def _build_nc():
    nc = bacc_mod.Bacc()

    # rhs packed per group of GP chunk-pairs: 64 partition-lines = {A-chunk
    # rows, A rows again, B rows, B rows again} x (pair cols side by side).
    rhs_d = nc.dram_tensor("rhs", [NGROUP, 64, GP * CHUNK], F16, kind="ExternalInput")
    # full K=32-padded weights for the four row-group bands (zeros included)
    lhsT_d = nc.dram_tensor("lhsT", [128, 128], F16, kind="ExternalInput")
    y_d = nc.dram_tensor("y", [NP], F32, kind="ExternalOutput")

    with TileContext(nc) as tc:
        with (
            tc.tile_pool(name="const", bufs=1) as constp,
            tc.tile_pool(name="cbp", bufs=PIPE_D + 5) as cbp,
            tc.tile_pool(name="ycp", bufs=3) as ycp,
            tc.tile_pool(name="psp", bufs=3, space="PSUM") as psp,
            tc.tile_pool(name="yp", bufs=2, space="PSUM") as yp,
        ):
            # --- constants ---
            lhsT_sb = constp.tile([128, 128], F16)
            nc.sync.dma_start(lhsT_sb[:], lhsT_d[:])
            ones_red = constp.tile([128, 32], F16)
            nc.vector.memset(ones_red[:], 1.0)
            scratch = constp.tile([128, CHUNK], F16)
            nc.vector.memset(scratch[:], 0.0)

            # static rhs buffers; odd 16-line bands are zeroed ONCE (they are
            # read by the K=32-padded matmuls against zero weight rows and
            # must not contain NaN junk)
            rhs_bufs = []
            for i in range(NRHSBUF):
                rb = constp.tile([128, GP * CHUNK], F16, name=f"rhsbuf{i}")
                nc.gpsimd.memset(rb[:], 0.0)
                rhs_bufs.append(rb)

            # --- HAM warm-up: row-tiled K=32 quads on memset data ---
            ps_w = psp.tile([128, 2 * CHUNK], F32, tag="ps")
            for _ in range(WARMUP_QUADS):
                for b in range(4):
                    nc.tensor.matmul(
                        ps_w[:, 256 * b : 256 * (b + 1)],
                        scratch[32 * b : 32 * b + 32, 0:128],
                        scratch[32 * b : 32 * b + 32, 0:256],
                        start=True,
                        stop=True,
                        tile_position=(32 * b, 0),
                    )

            cbs = {}  # chunk k -> cb fp16 AP
            state = {"yps": None, "next_red": 0}

            def reduce_quad(j):
                """ones-reduce for chunks 4j..4j+3, quad-concurrent."""
                yps = yp.tile([128, CHUNK], F32, tag="yps", name=f"yps_{j}")
                state["yps"] = yps
                quad = [cbs.pop(4 * j + q) for q in range(4)]
                for h in range(2):  # half 0 then half 1 (accumulate)
                    for q in range(4):
                        nc.tensor.matmul(
                            yps[32 * q : 32 * q + 32, :],
                            ones_red[:],
                            quad[q][:, h * CHUNK : (h + 1) * CHUNK],
                            start=(h == 0),
                            stop=(h == 1),
                            tile_position=(0, 32 * q),
                        )
                yc = ycp.tile([128, CHUNK], F32, tag="yc")
                if j % 2 == 0:
                    nc.scalar.copy(yc[:], yps[:])
                else:
                    nc.vector.tensor_copy(yc[:], yps[:])
                nc.gpsimd.dma_start(
                    y_d[4 * j * CHUNK : (4 * j + 4) * CHUNK].rearrange(
                        "(p f) -> p f", p=4
                    ),
                    yc[0:97:32, :],
                )

            def exp_stage(k, ps, lo):
                """exp of chunk k from psum tile ps columns [lo, lo+1024)."""
                if ACT_PATTERN[k % len(ACT_PATTERN)]:
                    cb = cbp.tile([128, 2 * CHUNK], F16, tag="cb", name=f"cb_{k}")
                    nc.scalar.activation(
                        cb[:],
                        ps[:, lo : lo + 2 * CHUNK],
                        mybir.ActivationFunctionType.Exp,
                        scale=LN2,
                    )
                    cbs[k] = cb[:]
                else:
                    cb = cbp.tile([128, 2 * CHUNK], U16, tag="cb", name=f"cb_{k}")
                    nc.vector.tensor_scalar(
                        cb[:],
                        ps[:, lo : lo + 2 * CHUNK],
                        1024.0,
                        SCH_BIAS,
                        mybir.AluOpType.mult,
                        mybir.AluOpType.add,
                    )
                    cbs[k] = cb[:].bitcast(F16)

            for p in range(NPAIR):
                gi, jj = divmod(p, GP)
                if jj == 0:
                    gsz = min(GP, NPAIR - gi * GP)
                    rhs_t = rhs_bufs[gi % NRHSBUF]
                    for b, eng in (
                        (0, nc.sync),
                        (1, nc.gpsimd),
                        (2, nc.sync),
                        (3, nc.gpsimd),
                    ):
                        eng.dma_start(
                            rhs_t[32 * b : 32 * b + 16, 0 : gsz * CHUNK],
                            rhs_d[gi, 16 * b : 16 * b + 16, 0 : gsz * CHUNK],
                        )
                col = jj * CHUNK
                kA, kB = 2 * p, 2 * p + 1
                psA = psp.tile([128, 2 * CHUNK], F32, tag="ps", name=f"psA_{p}")
                psB = psp.tile([128, 2 * CHUNK], F32, tag="ps", name=f"psB_{p}")
                # quad: (A,h0)->band0, (A,h1)->band32, (B,h0)->band64,
                # (B,h1)->band96 -- concurrent row-group tiles
                for b, ps, lo in (
                    (0, psA, 0),
                    (32, psA, CHUNK),
                    (64, psB, 0),
                    (96, psB, CHUNK),
                ):
                    nc.tensor.matmul(
                        ps[:, lo : lo + CHUNK],
                        lhsT_sb[b : b + 32, :],
                        rhs_t[b : b + 32, col : col + CHUNK],
                        start=True,
                        stop=True,
                        tile_position=(b, 0),
                    )

                exp_stage(kA, psA, 0)
                exp_stage(kB, psB, 0)

                while (
                    state["next_red"] * 4 + 3 <= kB - PIPE_D
                    and state["next_red"] * 4 + 3 < NCHUNK
                ):
                    reduce_quad(state["next_red"])
                    state["next_red"] += 1

            while state["next_red"] < NCHUNK // 4:
                reduce_quad(state["next_red"])
                state["next_red"] += 1
    nc.compile()
    return nc


def _host_prep(x, centers, coefficients):
    """O(M) center prep + per-core x layout, all in log2 units."""
    x = np.ascontiguousarray(np.asarray(x, dtype=np.float32))
    centers = np.asarray(centers, dtype=np.float32)
    coefficients = np.asarray(coefficients, dtype=np.float32)

    norm_const = np.float32(1.0 / ((2.0 * math.pi) ** (D / 2) * SIGMA**D))
    e = np.exp(coefficients - coefficients.max())
    w = (e / e.sum()).astype(np.float32)

    s = np.float32(math.sqrt(LOG2E))
    b = centers.T * s  # [4, 256]
    b_hi = b.astype(np.float16)
    b_lo = (b - b_hi.astype(np.float32)).astype(np.float16)

    g_raw = (
        np.log2(w * norm_const) - 0.5 * LOG2E * (centers**2).sum(axis=1)
    ).astype(np.float32)
    S = np.float32(math.floor(12.0 - np.log2(w * norm_const).max()))
    g = g_raw + S
    g_hi = g.astype(np.float16)
    g_lo = (g - g_hi.astype(np.float32)).astype(np.float16)

    halfw = np.zeros((2, 16, 128), dtype=np.float16)
    for h in range(2):
        sl = slice(128 * h, 128 * (h + 1))
        halfw[h, 0:4] = b_hi[:, sl]
        halfw[h, 4:8] = b_hi[:, sl]
        halfw[h, 8:12] = b_lo[:, sl]
        halfw[h, 12] = 1.0
        halfw[h, 13] = 1.0
        halfw[h, 14] = g_hi[sl]
        halfw[h, 15] = g_lo[sl]
    lhsT = np.zeros((128, 128), dtype=np.float16)
    lhsT[0:16] = halfw[0]
    lhsT[32:48] = halfw[1]
    lhsT[64:80] = halfw[0]
    lhsT[96:112] = halfw[1]

    in_maps = []
    for i in range(N_CORES):
        xs = x[i * PER_CORE : (i + 1) * PER_CORE]
        xp = np.zeros((NP, D), dtype=np.float32)
        xp[:PER_CORE] = xs
        a = xp * s
        a_hi = a.astype(np.float16)
        a_lo = (a - a_hi.astype(np.float32)).astype(np.float16)
        hbias = (-0.5 * LOG2E * (xp**2).sum(axis=1)).astype(np.float32)
        h_hi = hbias.astype(np.float16)
        h_lo = (hbias - h_hi.astype(np.float32)).astype(np.float16)
        rows = np.empty((16, NP), dtype=np.float16)
        rows[0:4] = a_hi.T
        rows[4:8] = a_lo.T
        rows[8:12] = a_hi.T
        rows[12] = h_hi
        rows[13] = h_lo
        rows[14] = 1.0
        rows[15] = 1.0
        # pack per group of GP pairs: 64 lines = {A rows, A rows, B rows,
        # B rows}, pair columns side by side within a line
        rc = rows.reshape(16, NCHUNK, CHUNK).transpose(1, 0, 2)  # [124,16,512]
        rp = rc.reshape(NPAIR, 2, 16, CHUNK)
        rhs = np.zeros((NGROUP, 64, GP * CHUNK), dtype=np.float16)
        for gi in range(NGROUP):
            p0 = gi * GP
            gsz = min(GP, NPAIR - p0)
            for j in range(gsz):
                cs = slice(j * CHUNK, (j + 1) * CHUNK)
                rhs[gi, 0:16, cs] = rp[p0 + j, 0]
                rhs[gi, 16:32, cs] = rp[p0 + j, 0]
                rhs[gi, 32:48, cs] = rp[p0 + j, 1]
                rhs[gi, 48:64, cs] = rp[p0 + j, 1]
        in_maps.append({"rhs": rhs, "lhsT": lhsT.copy()})
    return in_maps, float(S)


last_result = None


def kernel(x, centers, coefficients):
    global last_result
    if "nc" not in _CACHE:
        _CACHE["nc"] = _build_nc()
    nc = _CACHE["nc"]
    in_maps, S = _host_prep(x, centers, coefficients)
    res = run_bass_kernel_spmd(nc, in_maps, core_ids=list(range(N_CORES)))
    last_result = res
    y = np.concatenate([r["y"][:PER_CORE] for r in res.results])
    return (y * np.float32(2.0 ** (-S))).astype(np.float32)


# revision 14
# speedup vs baseline: 1.7170x; 1.0379x over previous
"""Trainium2 Bass kernel for GaussianKernelLayer.

y[n] = sum_m softmax(coef)[m] * norm * exp(-0.5*|x_n - c_m|^2),
N=500000, M=256, D=4, sigma=1. Data-parallel over 8 cores (x sharded on N).

v5 design (per core, NP=63488 padded rows, 124 chunks of 512):
  - One K=32 fp16 matmul per (chunk, m-half) assembles the full exp
    argument in log2 units directly in PSUM:
      psum[m, n] = log2e*(x.c - 0.5|x|^2) + [log2(w_m*norm)
                   - 0.5*log2e*|c_m|^2 + S]  =: t'   (gauss = 2^t')
    K is padded 16->32 with zero weight rows: the TRN2 HAM clock-gate
    only un-throttles the PE to 2.4 GHz when the 128x128 array is
    near-fully active, and 4 concurrent K=32 row-group tiles qualify
    (4 dense K=16 streams never warm up - measured).
  - Matmuls are issued in QUADS of adjacent instructions that target
    the 4 row-groups (mains: 2 chunks x 2 m-halves) or the 4
    col-groups (ones-reduce over m, K=128) -> each quad executes
    concurrently in ~215 ns warm (measured 209-211 ns/quad).
  - exp split across two engines, alternating by chunk:
      ACT: activation(Exp, scale=ln2) -> exact 2^t' (fp16)
      DVE: Schraudolph in ONE tensor_scalar: uint16(t'*1024+15*1024-C),
           the bit pattern IS fp16(2^t') (piecewise-linear mantissa).
  - Reduce stage is software-pipelined D chunks behind the mains so the
    in-order PE queue never stalls; rhs DMAs land in 4 row-group bands
    of static SBUF buffers (64 partition-lines per group DMA).
  - Host does O(M) prep, fp16 hi/lo splits, and the final 2^-S scale.
"""

import math

import numpy as np

import concourse.bass as bass
import concourse.bacc as bacc_mod
import concourse.mybir as mybir
from concourse.bass_utils import run_bass_kernel_spmd
from concourse.tile import TileContext

N_CORES = 8
N_TOTAL = 500000
PER_CORE = N_TOTAL // N_CORES  # 62500
CHUNK = 512
NCHUNK = 124
NP = CHUNK * NCHUNK  # 63488
M = 256
D = 4
SIGMA = 1.0

F16 = mybir.dt.float16
F32 = mybir.dt.float32
U16 = mybir.dt.uint16

LOG2E = 1.0 / math.log(2.0)
LN2 = math.log(2.0)
SCH_C = 60.0  # Schraudolph shift, tuned on host sim
SCH_BIAS = float(15 * 1024 - SCH_C)

# chunk -> exp engine (1 = ACT exact exp, 0 = DVE Schraudolph);
# pair-aware 8-chunk cycle: (A,D)(A,D)(A,D)(A,A) -> ACT 5/8, DVE 3/8
ACT_PATTERN = (1, 0, 1, 0, 1, 0, 1, 1)

PIPE_D = 6  # reduce stage lags the matmul stage by this many chunks
WARMUP_QUADS = 0
GP = 4  # chunk-PAIRS per rhs DMA group (8 chunks)
NPAIR = NCHUNK // 2  # 62
NGROUP = (NPAIR + GP - 1) // GP  # 16
NRHSBUF = 4

_CACHE = {}


def _build_nc():
    nc = bacc_mod.Bacc()

    # rhs packed per group of GP chunk-pairs: 64 partition-lines = {A-chunk
    # rows, A rows again, B rows, B rows again} x (pair cols side by side).
    rhs_d = nc.dram_tensor("rhs", [NGROUP, 64, GP * CHUNK], F16, kind="ExternalInput")
    # full K=32-padded weights for the four row-group bands (zeros included)
    lhsT_d = nc.dram_tensor("lhsT", [128, 128], F16, kind="ExternalInput")
    y_d = nc.dram_tensor("y", [NP], F32, kind="ExternalOutput")

    with TileContext(nc) as tc:
        with (
            tc.tile_pool(name="const", bufs=1) as constp,
            tc.tile_pool(name="cbp", bufs=PIPE_D + 5) as cbp,
            tc.tile_pool(name="ycp", bufs=3) as ycp,
            tc.tile_pool(name="psp", bufs=3, space="PSUM") as psp,
            tc.tile_pool(name="yp", bufs=2, space="PSUM") as yp,
        ):
            # --- constants ---
            lhsT_sb = constp.tile([128, 128], F16)
            nc.sync.dma_start(lhsT_sb[:], lhsT_d[:])
            ones_red = constp.tile([128, 32], F16)
            nc.vector.memset(ones_red[:], 1.0)
            scratch = constp.tile([128, CHUNK], F16)
            nc.vector.memset(scratch[:], 0.0)

            # static rhs buffers; odd 16-line bands are zeroed ONCE (they are
            # read by the K=32-padded matmuls against zero weight rows and
            # must not contain NaN junk)
            rhs_bufs = []
            for i in range(NRHSBUF):
                rb = constp.tile([128, GP * CHUNK], F16, name=f"rhsbuf{i}")
                nc.vector.memset(rb[:], 0.0)
                rhs_bufs.append(rb)

            # --- HAM warm-up: row-tiled K=32 quads on memset data ---
            ps_w = psp.tile([128, 2 * CHUNK], F32, tag="ps")
            for _ in range(WARMUP_QUADS):
                for b in range(4):
                    nc.tensor.matmul(
                        ps_w[:, 256 * b : 256 * (b + 1)],
                        scratch[32 * b : 32 * b + 32, 0:128],
                        scratch[32 * b : 32 * b + 32, 0:256],
                        start=True,
                        stop=True,
                        tile_position=(32 * b, 0),
                    )

            cbs = {}  # chunk k -> cb fp16 AP
            state = {"yps": None, "next_red": 0}

            def reduce_quad(j):
                """ones-reduce for chunks 4j..4j+3, quad-concurrent."""
                yps = yp.tile([128, CHUNK], F32, tag="yps", name=f"yps_{j}")
                state["yps"] = yps
                quad = [cbs.pop(4 * j + q) for q in range(4)]
                for h in range(2):  # half 0 then half 1 (accumulate)
                    for q in range(4):
                        nc.tensor.matmul(
                            yps[32 * q : 32 * q + 32, :],
                            ones_red[:],
                            quad[q][:, h * CHUNK : (h + 1) * CHUNK],
                            start=(h == 0),
                            stop=(h == 1),
                            tile_position=(0, 32 * q),
                        )
                yc = ycp.tile([128, CHUNK], F32, tag="yc")
                nc.vector.tensor_copy(yc[:], yps[:])
                nc.gpsimd.dma_start(
                    y_d[4 * j * CHUNK : (4 * j + 4) * CHUNK].rearrange(
                        "(p f) -> p f", p=4
                    ),
                    yc[0:97:32, :],
                )

            def exp_stage(k, ps, lo):
                """exp of chunk k from psum tile ps columns [lo, lo+1024)."""
                if ACT_PATTERN[k % len(ACT_PATTERN)]:
                    cb = cbp.tile([128, 2 * CHUNK], F16, tag="cb", name=f"cb_{k}")
                    nc.scalar.activation(
                        cb[:],
                        ps[:, lo : lo + 2 * CHUNK],
                        mybir.ActivationFunctionType.Exp,
                        scale=LN2,
                    )
                    cbs[k] = cb[:]
                else:
                    cb = cbp.tile([128, 2 * CHUNK], U16, tag="cb", name=f"cb_{k}")
                    nc.vector.tensor_scalar(
                        cb[:],
                        ps[:, lo : lo + 2 * CHUNK],
                        1024.0,
                        SCH_BIAS,
                        mybir.AluOpType.mult,
                        mybir.AluOpType.add,
                    )
                    cbs[k] = cb[:].bitcast(F16)

            for p in range(NPAIR):
                gi, jj = divmod(p, GP)
                if jj == 0:
                    gsz = min(GP, NPAIR - gi * GP)
                    rhs_t = rhs_bufs[gi % NRHSBUF]
                    for b, eng in (
                        (0, nc.sync),
                        (1, nc.gpsimd),
                        (2, nc.sync),
                        (3, nc.gpsimd),
                    ):
                        eng.dma_start(
                            rhs_t[32 * b : 32 * b + 16, 0 : gsz * CHUNK],
                            rhs_d[gi, 16 * b : 16 * b + 16, 0 : gsz * CHUNK],
                        )
                col = jj * CHUNK
                kA, kB = 2 * p, 2 * p + 1
                psA = psp.tile([128, 2 * CHUNK], F32, tag="ps", name=f"psA_{p}")
                psB = psp.tile([128, 2 * CHUNK], F32, tag="ps", name=f"psB_{p}")
                # quad: (A,h0)->band0, (A,h1)->band32, (B,h0)->band64,
                # (B,h1)->band96 -- concurrent row-group tiles
                for b, ps, lo in (
                    (0, psA, 0),
                    (32, psA, CHUNK),
                    (64, psB, 0),
                    (96, psB, CHUNK),
                ):
                    nc.tensor.matmul(
                        ps[:, lo : lo + CHUNK],
                        lhsT_sb[b : b + 32, :],
                        rhs_t[b : b + 32, col : col + CHUNK],
                        start=True,
                        stop=True,
                        tile_position=(b, 0),
                    )

                exp_stage(kA, psA, 0)
                exp_stage(kB, psB, 0)

                while (
                    state["next_red"] * 4 + 3 <= kB - PIPE_D
                    and state["next_red"] * 4 + 3 < NCHUNK
                ):
                    reduce_quad(state["next_red"])
                    state["next_red"] += 1

            while state["next_red"] < NCHUNK // 4:
                reduce_quad(state["next_red"])
                state["next_red"] += 1
    nc.compile()
    return nc


def _host_prep(x, centers, coefficients):
    """O(M) center prep + per-core x layout, all in log2 units."""
    x = np.ascontiguousarray(np.asarray(x, dtype=np.float32))
    centers = np.asarray(centers, dtype=np.float32)
    coefficients = np.asarray(coefficients, dtype=np.float32)

    norm_const = np.float32(1.0 / ((2.0 * math.pi) ** (D / 2) * SIGMA**D))
    e = np.exp(coefficients - coefficients.max())
    w = (e / e.sum()).astype(np.float32)

    s = np.float32(math.sqrt(LOG2E))
    b = centers.T * s  # [4, 256]
    b_hi = b.astype(np.float16)
    b_lo = (b - b_hi.astype(np.float32)).astype(np.float16)

    g_raw = (
        np.log2(w * norm_const) - 0.5 * LOG2E * (centers**2).sum(axis=1)
    ).astype(np.float32)
    S = np.float32(math.floor(12.0 - np.log2(w * norm_const).max()))
    g = g_raw + S
    g_hi = g.astype(np.float16)
    g_lo = (g - g_hi.astype(np.float32)).astype(np.float16)

    halfw = np.zeros((2, 16, 128), dtype=np.float16)
    for h in range(2):
        sl = slice(128 * h, 128 * (h + 1))
        halfw[h, 0:4] = b_hi[:, sl]
        halfw[h, 4:8] = b_hi[:, sl]
        halfw[h, 8:12] = b_lo[:, sl]
        halfw[h, 12] = 1.0
        halfw[h, 13] = 1.0
        halfw[h, 14] = g_hi[sl]
        halfw[h, 15] = g_lo[sl]
    lhsT = np.zeros((128, 128), dtype=np.float16)
    lhsT[0:16] = halfw[0]
    lhsT[32:48] = halfw[1]
    lhsT[64:80] = halfw[0]
    lhsT[96:112] = halfw[1]

    in_maps = []
    for i in range(N_CORES):
        xs = x[i * PER_CORE : (i + 1) * PER_CORE]
        xp = np.zeros((NP, D), dtype=np.float32)
        xp[:PER_CORE] = xs
        a = xp * s
        a_hi = a.astype(np.float16)
        a_lo = (a - a_hi.astype(np.float32)).astype(np.float16)
        hbias = (-0.5 * LOG2E * (xp**2).sum(axis=1)).astype(np.float32)
        h_hi = hbias.astype(np.float16)
        h_lo = (hbias - h_hi.astype(np.float32)).astype(np.float16)
        rows = np.empty((16, NP), dtype=np.float16)
        rows[0:4] = a_hi.T
        rows[4:8] = a_lo.T
        rows[8:12] = a_hi.T
        rows[12] = h_hi
        rows[13] = h_lo
        rows[14] = 1.0
        rows[15] = 1.0
        # pack per group of GP pairs: 64 lines = {A rows, A rows, B rows,
        # B rows}, pair columns side by side within a line
        rc = rows.reshape(16, NCHUNK, CHUNK).transpose(1, 0, 2)  # [124,16,512]
        rp = rc.reshape(NPAIR, 2, 16, CHUNK)
        rhs = np.zeros((NGROUP, 64, GP * CHUNK), dtype=np.float16)
        for gi in range(NGROUP):
            p0 = gi * GP
            gsz = min(GP, NPAIR - p0)
            for j in range(gsz):
                cs = slice(j * CHUNK, (j + 1) * CHUNK)
                rhs[gi, 0:16, cs] = rp[p0 + j, 0]
                rhs[gi, 16:32, cs] = rp[p0 + j, 0]
                rhs[gi, 32:48, cs] = rp[p0 + j, 1]
                rhs[gi, 48:64, cs] = rp[p0 + j, 1]
        in_maps.append({"rhs": rhs, "lhsT": lhsT.copy()})
    return in_maps, float(S)


last_result = None


def kernel(x, centers, coefficients):
    global last_result
    if "nc" not in _CACHE:
        _CACHE["nc"] = _build_nc()
    nc = _CACHE["nc"]
    in_maps, S = _host_prep(x, centers, coefficients)
    res = run_bass_kernel_spmd(nc, in_maps, core_ids=list(range(N_CORES)))
    last_result = res
    y = np.concatenate([r["y"][:PER_CORE] for r in res.results])
    return (y * np.float32(2.0 ** (-S))).astype(np.float32)


# revision 15
# speedup vs baseline: 1.7211x; 1.0024x over previous
"""Trainium2 Bass kernel for GaussianKernelLayer.

y[n] = sum_m softmax(coef)[m] * norm * exp(-0.5*|x_n - c_m|^2),
N=500000, M=256, D=4, sigma=1. Data-parallel over 8 cores (x sharded on N).

v5 design (per core, NP=63488 padded rows, 124 chunks of 512):
  - One K=32 fp16 matmul per (chunk, m-half) assembles the full exp
    argument in log2 units directly in PSUM:
      psum[m, n] = log2e*(x.c - 0.5|x|^2) + [log2(w_m*norm)
                   - 0.5*log2e*|c_m|^2 + S]  =: t'   (gauss = 2^t')
    K is padded 16->32 with zero weight rows: the TRN2 HAM clock-gate
    only un-throttles the PE to 2.4 GHz when the 128x128 array is
    near-fully active, and 4 concurrent K=32 row-group tiles qualify
    (4 dense K=16 streams never warm up - measured).
  - Matmuls are issued in QUADS of adjacent instructions that target
    the 4 row-groups (mains: 2 chunks x 2 m-halves) or the 4
    col-groups (ones-reduce over m, K=128) -> each quad executes
    concurrently in ~215 ns warm (measured 209-211 ns/quad).
  - exp split across two engines, alternating by chunk:
      ACT: activation(Exp, scale=ln2) -> exact 2^t' (fp16)
      DVE: Schraudolph in ONE tensor_scalar: uint16(t'*1024+15*1024-C),
           the bit pattern IS fp16(2^t') (piecewise-linear mantissa).
  - Reduce stage is software-pipelined D chunks behind the mains so the
    in-order PE queue never stalls; rhs DMAs land in 4 row-group bands
    of static SBUF buffers (64 partition-lines per group DMA).
  - Host does O(M) prep, fp16 hi/lo splits, and the final 2^-S scale.
"""

import math

import numpy as np

import concourse.bass as bass
import concourse.bacc as bacc_mod
import concourse.mybir as mybir
from concourse.bass_utils import run_bass_kernel_spmd
from concourse.tile import TileContext

N_CORES = 8
N_TOTAL = 500000
PER_CORE = N_TOTAL // N_CORES  # 62500
CHUNK = 512
NCHUNK = 124
NP = CHUNK * NCHUNK  # 63488
M = 256
D = 4
SIGMA = 1.0

F16 = mybir.dt.float16
F32 = mybir.dt.float32
U16 = mybir.dt.uint16

LOG2E = 1.0 / math.log(2.0)
LN2 = math.log(2.0)
SCH_C = 60.0  # Schraudolph shift, tuned on host sim
SCH_BIAS = float(15 * 1024 - SCH_C)

# chunk -> exp engine (1 = ACT exact exp, 0 = DVE Schraudolph);
# pair-aware 16-chunk cycle: 7x(A,D) + (A,A) -> ACT 9/16, DVE 7/16
ACT_PATTERN = (1, 0, 1, 0, 1, 0, 1, 0, 1, 0, 1, 0, 1, 0, 1, 1)

PIPE_D = 6  # reduce stage lags the matmul stage by this many chunks
WARMUP_MM = 12
GP = 4  # chunk-PAIRS per rhs DMA group (8 chunks)
NPAIR = NCHUNK // 2  # 62
NGROUP = (NPAIR + GP - 1) // GP  # 16
NRHSBUF = 4

_CACHE = {}


def _build_nc():
    nc = bacc_mod.Bacc()

    # rhs packed per group of GP chunk-pairs: 64 partition-lines = {A-chunk
    # rows, A rows again, B rows, B rows again} x (pair cols side by side).
    rhs_d = nc.dram_tensor("rhs", [NGROUP, 64, GP * CHUNK], F16, kind="ExternalInput")
    # full K=32-padded weights for the four row-group bands (zeros included)
    lhsT_d = nc.dram_tensor("lhsT", [128, 128], F16, kind="ExternalInput")
    y_d = nc.dram_tensor("y", [NP], F32, kind="ExternalOutput")

    with TileContext(nc) as tc:
        with (
            tc.tile_pool(name="const", bufs=1) as constp,
            tc.tile_pool(name="cbp", bufs=PIPE_D + 5) as cbp,
            tc.tile_pool(name="ycp", bufs=3) as ycp,
            tc.tile_pool(name="psp", bufs=3, space="PSUM") as psp,
            tc.tile_pool(name="yp", bufs=2, space="PSUM") as yp,
        ):
            # --- constants ---
            lhsT_sb = constp.tile([128, 128], F16)
            nc.sync.dma_start(lhsT_sb[:], lhsT_d[:])
            ones_red = constp.tile([128, 32], F16)
            nc.vector.memset(ones_red[:], 1.0)
            scratch = constp.tile([128, CHUNK], F16)
            nc.vector.memset(scratch[:], 0.0)

            # static rhs buffers; odd 16-line bands are zeroed ONCE (they are
            # read by the K=32-padded matmuls against zero weight rows and
            # must not contain NaN junk)
            rhs_bufs = []
            for i in range(NRHSBUF):
                rb = constp.tile([128, GP * CHUNK], F16, name=f"rhsbuf{i}")
                nc.vector.memset(rb[:], 0.0)
                rhs_bufs.append(rb)

            # --- HAM warm-up: serial full-array K=128 matmuls on memset
            # data (the one shape measured to flip the clock-gate to 2.4
            # GHz; once warm it stays warm - re-throttle needs ~3.4us of
            # CONTIGUOUS PE idle, which the steady state never has) ---
            ps_w = psp.tile([128, 2 * CHUNK], F32, tag="ps")
            for i in range(WARMUP_MM):
                nc.tensor.matmul(
                    ps_w[:, CHUNK * (i % 2) : CHUNK * (i % 2 + 1)],
                    scratch[:, 0:128],
                    scratch[:, 0:CHUNK],
                    start=True,
                    stop=True,
                )

            cbs = {}  # chunk k -> cb fp16 AP
            state = {"yps": None, "next_red": 0}

            def reduce_quad(j):
                """ones-reduce for chunks 4j..4j+3, quad-concurrent."""
                yps = yp.tile([128, CHUNK], F32, tag="yps", name=f"yps_{j}")
                state["yps"] = yps
                quad = [cbs.pop(4 * j + q) for q in range(4)]
                for h in range(2):  # half 0 then half 1 (accumulate)
                    for q in range(4):
                        nc.tensor.matmul(
                            yps[32 * q : 32 * q + 32, :],
                            ones_red[:],
                            quad[q][:, h * CHUNK : (h + 1) * CHUNK],
                            start=(h == 0),
                            stop=(h == 1),
                            tile_position=(0, 32 * q),
                        )
                yc = ycp.tile([128, CHUNK], F32, tag="yc")
                nc.vector.tensor_copy(yc[:], yps[:])
                nc.gpsimd.dma_start(
                    y_d[4 * j * CHUNK : (4 * j + 4) * CHUNK].rearrange(
                        "(p f) -> p f", p=4
                    ),
                    yc[0:97:32, :],
                )

            def exp_stage(k, ps, lo):
                """exp of chunk k from psum tile ps columns [lo, lo+1024)."""
                if ACT_PATTERN[k % len(ACT_PATTERN)]:
                    cb = cbp.tile([128, 2 * CHUNK], F16, tag="cb", name=f"cb_{k}")
                    nc.scalar.activation(
                        cb[:],
                        ps[:, lo : lo + 2 * CHUNK],
                        mybir.ActivationFunctionType.Exp,
                        scale=LN2,
                    )
                    cbs[k] = cb[:]
                else:
                    cb = cbp.tile([128, 2 * CHUNK], U16, tag="cb", name=f"cb_{k}")
                    nc.vector.tensor_scalar(
                        cb[:],
                        ps[:, lo : lo + 2 * CHUNK],
                        1024.0,
                        SCH_BIAS,
                        mybir.AluOpType.mult,
                        mybir.AluOpType.add,
                    )
                    cbs[k] = cb[:].bitcast(F16)

            for p in range(NPAIR):
                gi, jj = divmod(p, GP)
                if jj == 0:
                    gsz = min(GP, NPAIR - gi * GP)
                    rhs_t = rhs_bufs[gi % NRHSBUF]
                    for b, eng in (
                        (0, nc.sync),
                        (1, nc.gpsimd),
                        (2, nc.sync),
                        (3, nc.gpsimd),
                    ):
                        eng.dma_start(
                            rhs_t[32 * b : 32 * b + 16, 0 : gsz * CHUNK],
                            rhs_d[gi, 16 * b : 16 * b + 16, 0 : gsz * CHUNK],
                        )
                col = jj * CHUNK
                kA, kB = 2 * p, 2 * p + 1
                psA = psp.tile([128, 2 * CHUNK], F32, tag="ps", name=f"psA_{p}")
                psB = psp.tile([128, 2 * CHUNK], F32, tag="ps", name=f"psB_{p}")
                # quad: (A,h0)->band0, (A,h1)->band32, (B,h0)->band64,
                # (B,h1)->band96 -- concurrent row-group tiles
                for b, ps, lo in (
                    (0, psA, 0),
                    (32, psA, CHUNK),
                    (64, psB, 0),
                    (96, psB, CHUNK),
                ):
                    nc.tensor.matmul(
                        ps[:, lo : lo + CHUNK],
                        lhsT_sb[b : b + 32, :],
                        rhs_t[b : b + 32, col : col + CHUNK],
                        start=True,
                        stop=True,
                        tile_position=(b, 0),
                    )

                exp_stage(kA, psA, 0)
                exp_stage(kB, psB, 0)

                while (
                    state["next_red"] * 4 + 3 <= kB - PIPE_D
                    and state["next_red"] * 4 + 3 < NCHUNK
                ):
                    reduce_quad(state["next_red"])
                    state["next_red"] += 1

            while state["next_red"] < NCHUNK // 4:
                reduce_quad(state["next_red"])
                state["next_red"] += 1
    nc.compile()
    return nc


def _host_prep(x, centers, coefficients):
    """O(M) center prep + per-core x layout, all in log2 units."""
    x = np.ascontiguousarray(np.asarray(x, dtype=np.float32))
    centers = np.asarray(centers, dtype=np.float32)
    coefficients = np.asarray(coefficients, dtype=np.float32)

    norm_const = np.float32(1.0 / ((2.0 * math.pi) ** (D / 2) * SIGMA**D))
    e = np.exp(coefficients - coefficients.max())
    w = (e / e.sum()).astype(np.float32)

    s = np.float32(math.sqrt(LOG2E))
    b = centers.T * s  # [4, 256]
    b_hi = b.astype(np.float16)
    b_lo = (b - b_hi.astype(np.float32)).astype(np.float16)

    g_raw = (
        np.log2(w * norm_const) - 0.5 * LOG2E * (centers**2).sum(axis=1)
    ).astype(np.float32)
    S = np.float32(math.floor(12.0 - np.log2(w * norm_const).max()))
    g = g_raw + S
    g_hi = g.astype(np.float16)
    g_lo = (g - g_hi.astype(np.float32)).astype(np.float16)

    halfw = np.zeros((2, 16, 128), dtype=np.float16)
    for h in range(2):
        sl = slice(128 * h, 128 * (h + 1))
        halfw[h, 0:4] = b_hi[:, sl]
        halfw[h, 4:8] = b_hi[:, sl]
        halfw[h, 8:12] = b_lo[:, sl]
        halfw[h, 12] = 1.0
        halfw[h, 13] = 1.0
        halfw[h, 14] = g_hi[sl]
        halfw[h, 15] = g_lo[sl]
    lhsT = np.zeros((128, 128), dtype=np.float16)
    lhsT[0:16] = halfw[0]
    lhsT[32:48] = halfw[1]
    lhsT[64:80] = halfw[0]
    lhsT[96:112] = halfw[1]

    in_maps = []
    for i in range(N_CORES):
        xs = x[i * PER_CORE : (i + 1) * PER_CORE]
        xp = np.zeros((NP, D), dtype=np.float32)
        xp[:PER_CORE] = xs
        a = xp * s
        a_hi = a.astype(np.float16)
        a_lo = (a - a_hi.astype(np.float32)).astype(np.float16)
        hbias = (-0.5 * LOG2E * (xp**2).sum(axis=1)).astype(np.float32)
        h_hi = hbias.astype(np.float16)
        h_lo = (hbias - h_hi.astype(np.float32)).astype(np.float16)
        rows = np.empty((16, NP), dtype=np.float16)
        rows[0:4] = a_hi.T
        rows[4:8] = a_lo.T
        rows[8:12] = a_hi.T
        rows[12] = h_hi
        rows[13] = h_lo
        rows[14] = 1.0
        rows[15] = 1.0
        # pack per group of GP pairs: 64 lines = {A rows, A rows, B rows,
        # B rows}, pair columns side by side within a line
        rc = rows.reshape(16, NCHUNK, CHUNK).transpose(1, 0, 2)  # [124,16,512]
        rp = rc.reshape(NPAIR, 2, 16, CHUNK)
        rhs = np.zeros((NGROUP, 64, GP * CHUNK), dtype=np.float16)
        for gi in range(NGROUP):
            p0 = gi * GP
            gsz = min(GP, NPAIR - p0)
            for j in range(gsz):
                cs = slice(j * CHUNK, (j + 1) * CHUNK)
                rhs[gi, 0:16, cs] = rp[p0 + j, 0]
                rhs[gi, 16:32, cs] = rp[p0 + j, 0]
                rhs[gi, 32:48, cs] = rp[p0 + j, 1]
                rhs[gi, 48:64, cs] = rp[p0 + j, 1]
        in_maps.append({"rhs": rhs, "lhsT": lhsT.copy()})
    return in_maps, float(S)


last_result = None


def kernel(x, centers, coefficients):
    global last_result
    if "nc" not in _CACHE:
        _CACHE["nc"] = _build_nc()
    nc = _CACHE["nc"]
    in_maps, S = _host_prep(x, centers, coefficients)
    res = run_bass_kernel_spmd(nc, in_maps, core_ids=list(range(N_CORES)))
    last_result = res
    y = np.concatenate([r["y"][:PER_CORE] for r in res.results])
    return (y * np.float32(2.0 ** (-S))).astype(np.float32)
